# revision 23
# baseline (speedup 1.0000x reference)
"""Distance-biased FAVOR+ fast attention on 8 Trainium2 NeuronCores.

Strategy: shard the 32 (batch, head) pairs across 8 cores (4 pairs/core).
Per pair, the device computes:
    qs'^T = exp(W~ @ Xq~^T)    (transposed layout, m on partitions)  [bf16]
    ks'   = exp(Xk~ @ W~^T)    (natural layout, L on partitions)     [bf16]
    buf1  = ks'^T @ c          (PSUM accumulation over 32 L-chunks)
    buf2  = qs' @ buf1         (natural layout), then num / max(den, EPS)
where Xq~/Xk~ are host-precomputed 73-dim augmented features
  [x*D^-0.25 (64), fourier dist features (8), h = 0.5*|x|^2 + ln(16)]
padded to 128 rows (full-partition DMAs run 5x faster than 73-row ones)
in fp16, and W~ = [qk_proj | -1] so exp(W~ X~^T) = exp(x W^T - h)/16 = phi(x).
The 1/sqrt(m) and -h terms ride inside the matmul; ACT exp needs no bias.
"""

import numpy as np
import ml_dtypes

B, L, H, D, DV = 4, 4096, 8, 64, 64
M = 256
DSH = 8             # per-head fourier feature dim (S // H)
KAUG = D + DSH + 1  # 73
NCORES = 8
PPC = (B * H) // NCORES  # pairs per core = 4
NCH = L // 128           # 32 chunks of 128 rows
EPS = 1e-6
LN16 = float(np.log(16.0))
TWO_PI = 2.0 * np.pi
NORM = float(D) ** -0.25

_CACHE = {}


def _build_kernel():
    import concourse.bass as bass
    import concourse.bacc as bacc
    import concourse.mybir as mybir
    import concourse.tile as tile

    f32 = mybir.dt.float32
    bf16 = mybir.dt.bfloat16
    fp16 = mybir.dt.float16
    Exp = mybir.ActivationFunctionType.Exp

    nc = bacc.Bacc("TRN2", debug=False, num_devices=NCORES)
    xq_t = nc.dram_tensor("xq_t", [PPC, 128, L], fp16, kind="ExternalInput")
    xk_t = nc.dram_tensor("xk_t", [PPC, 128, L], fp16, kind="ExternalInput")
    cb = nc.dram_tensor("cb", [PPC, 128, NCH, DV + 1], bf16, kind="ExternalInput")
    wt = nc.dram_tensor("wt", [128, M], fp16, kind="ExternalInput")
    out = nc.dram_tensor("out", [PPC, 128, NCH, DV], fp16, kind="ExternalOutput")

    with tile.TileContext(nc) as tc:
        with (
            tc.tile_pool(name="singles", bufs=1) as singles,
            tc.tile_pool(name="xin", bufs=3) as xin,
            tc.tile_pool(name="qp", bufs=2) as qp,
            tc.tile_pool(name="kpl", bufs=4) as kpl,
            tc.tile_pool(name="cpool", bufs=2) as cpool,
            tc.tile_pool(name="bsb", bufs=6) as bsb,
            tc.tile_pool(name="dvp", bufs=4) as dvp,
            tc.tile_pool(name="obp", bufs=3) as obp,
            tc.tile_pool(name="pslog", bufs=2, space="PSUM") as pslog,
            tc.tile_pool(name="psd", bufs=2, space="PSUM") as psd,
        ):
            wt_sb = singles.tile([128, M], fp16)
            nc.gpsimd.dma_start(out=wt_sb, in_=wt[:, :])

            for i in range(PPC):
                xk = xin.tile([128, L], fp16, tag="xk")
                nc.gpsimd.dma_start(out=xk[:, :L // 4], in_=xk_t[i, :, :L // 4])
                c_sb = cpool.tile([128, NCH, DV + 1], bf16)
                nc.gpsimd.dma_start(out=c_sb, in_=cb[i])
                for piece in range(1, 4):
                    nc.gpsimd.dma_start(
                        out=xk[:, L // 4 * piece:L // 4 * (piece + 1)],
                        in_=xk_t[i, :, L // 4 * piece:L // 4 * (piece + 1)])
                xq = xin.tile([128, L], fp16, tag="xq")
                for piece in range(4):
                    nc.gpsimd.dma_start(
                        out=xq[:, L // 4 * piece:L // 4 * (piece + 1)],
                        in_=xq_t[i, :, L // 4 * piece:L // 4 * (piece + 1)])

                # ---- phase A-k + C: ks' natural; buf1 += ks'^T @ c per chunk
                b1ps = [
                    psd.tile([128, 7 * (DV + 1)], f32, tag="dout",
                             name=f"b1_{i}_{mi}")[:, :DV + 1]
                    for mi in range(2)
                ]
                nchunk = 0
                for g, gch in enumerate((2, 6, 6, 6, 6, 6)):
                    ps = pslog.tile([128, 1536], f32, tag="logits", name=f"psk_{i}_{g}")
                    for j in range(gch):
                        n = nchunk + j
                        nc.tensor.matmul(
                            ps[:, 256 * j:256 * (j + 1)],
                            lhsT=xk[:, 128 * n:128 * (n + 1)],
                            rhs=wt_sb,
                            start=True, stop=True,
                        )
                    kp = kpl.tile([128, 1536], bf16, tag="kp", name=f"kp_{i}_{g}")
                    nc.scalar.activation(
                        out=kp[:, :256 * gch], in_=ps[:, :256 * gch], func=Exp,
                        bias=0.0, scale=1.0)
                    for j in range(gch):
                        n = nchunk + j
                        for mi in range(2):
                            nc.tensor.matmul(
                                b1ps[mi],
                                lhsT=kp[:, 256 * j + 128 * mi:256 * j + 128 * (mi + 1)],
                                rhs=c_sb[:, n, :],
                                start=(n == 0), stop=(n == NCH - 1),
                            )
                    nchunk += gch
                buf1 = []
                for mi in range(2):
                    bsl = bsb.tile([128, DV + 1], bf16, tag=f"b1sb_{mi}",
                                   name=f"b1sb_{i}_{mi}")
                    nc.vector.tensor_copy(out=bsl, in_=b1ps[mi])
                    buf1.append(bsl)

                # ---- phase A-q: qs'^T = exp(W~ Xq~^T); g outer, mi inner so
                # phase D's chunk groups unblock as early as possible
                qpT = [
                    qp.tile([128, L], bf16, tag=f"qpT{mi}", name=f"qpT{mi}_{i}")
                    for mi in range(2)
                ]
                col = 0
                for g, w in enumerate((1536, 1536, 1024)):
                    for mi in range(2):
                        ps = pslog.tile([128, 1536], f32, tag="logits",
                                        name=f"psq_{i}_{mi}_{g}")
                        for n2 in range(w // 512):
                            nc.tensor.matmul(
                                ps[:, 512 * n2:512 * (n2 + 1)],
                                lhsT=wt_sb[:, 128 * mi:128 * (mi + 1)],
                                rhs=xq[:, col + 512 * n2:col + 512 * (n2 + 1)],
                                start=True, stop=True,
                            )
                        nc.scalar.activation(
                            out=qpT[mi][:, col:col + w], in_=ps[:, :w],
                            func=Exp, bias=0.0, scale=1.0,
                        )
                    col += w

                # ---- phase D: buf2 = qs' @ buf1 per L-chunk; divide; store
                ob = obp.tile([128, NCH * DV], fp16, tag="ob", name=f"ob_{i}")
                n0 = 0
                for gi, cnt in enumerate((7, 7, 7, 7, 4)):
                    dps = psd.tile([128, 7 * (DV + 1)], f32, tag="dout",
                                   name=f"dps_{i}_{gi}")
                    dps3 = dps.rearrange("p (s e) -> p s e", e=DV + 1)
                    for s in range(cnt):
                        n = n0 + s
                        for mi in range(2):
                            nc.tensor.matmul(
                                dps3[:, s, :],
                                lhsT=qpT[mi][:, 128 * n:128 * (n + 1)],
                                rhs=buf1[mi],
                                start=(mi == 0), stop=(mi == 1),
                            )
                    den = dvp.tile([128, 7], f32, tag="den", name=f"den_{i}_{gi}")
                    nc.vector.tensor_scalar_max(
                        out=den[:, :cnt], in0=dps3[:, :cnt, DV], scalar1=EPS)
                    nc.vector.reciprocal(out=den[:, :cnt], in_=den[:, :cnt])
                    den_sl = den[:, :cnt]
                    den_bc = bass.AP(
                        tensor=den_sl.tensor, offset=den_sl.offset,
                        ap=[den_sl.ap[0], den_sl.ap[1], [0, DV]])
                    nc.vector.tensor_tensor(
                        out=ob[:, DV * n0:DV * (n0 + cnt)].rearrange(
                            "p (s e) -> p s e", e=DV),
                        in0=dps3[:, :cnt, 0:DV],
                        in1=den_bc,
                        op=mybir.AluOpType.mult,
                    )
                    oeng = nc.sync if i == PPC - 1 else nc.gpsimd
                    ob3 = ob.rearrange("p (n e) -> p n e", e=DV)
                    oeng.dma_start(out=out[i, :, n0:n0 + cnt, :],
                                   in_=ob3[:, n0:n0 + cnt, :])
                    n0 += cnt
    nc.compile()
    return nc


def _prep_inputs(qs, ks, vs, qs_s, ks_s, fourier_W, qk_proj, a):
    """Host-side: fourier features, augmentation, transposes, per-core split."""
    pq = TWO_PI * (qs_s @ fourier_W)       # (B, L, 32)
    pk = TWO_PI * (ks_s @ fourier_W)
    embq = np.concatenate([np.sin(pq), np.cos(pq)], axis=-1).astype(np.float32)
    embk = np.concatenate([np.sin(pk), np.cos(pk)], axis=-1).astype(np.float32)
    qs_sp = (a * embq.reshape(B, L, H, DSH)).astype(np.float32)  # (B,L,H,8)
    ks_sp = embk.reshape(B, L, H, DSH)

    xq = np.concatenate([qs * NORM, qs_sp], axis=-1)  # (B,L,H,72)
    xk = np.concatenate([ks * NORM, ks_sp], axis=-1)
    hq = 0.5 * np.sum(np.square(xq), axis=-1, keepdims=True) + LN16
    hk = 0.5 * np.sum(np.square(xk), axis=-1, keepdims=True) + LN16
    xq = np.concatenate([xq, hq], axis=-1)  # (B,L,H,73)
    xk = np.concatenate([xk, hk], axis=-1)

    c = np.concatenate([vs, np.ones((B, L, H, 1), vs.dtype)], axis=-1)
    c = c.astype(ml_dtypes.bfloat16)        # (B,L,H,65)

    wt_f = np.concatenate(
        [qk_proj, -np.ones((M, 1), np.float32)], axis=1).T  # (73, 256)
    wt = np.zeros((128, M), np.float16)
    wt[:KAUG] = wt_f

    pairs = [(b, h) for b in range(B) for h in range(H)]
    in_maps = []
    for core in range(NCORES):
        sel = pairs[PPC * core:PPC * (core + 1)]
        xq_t = np.zeros((PPC, 128, L), np.float16)
        xk_t = np.zeros((PPC, 128, L), np.float16)
        for j, (b, h) in enumerate(sel):
            xq_t[j, :KAUG] = xq[b, :, h, :].T
            xk_t[j, :KAUG] = xk[b, :, h, :].T
        cbs = np.stack([
            c[b, :, h, :].reshape(NCH, 128, DV + 1).transpose(1, 0, 2)
            for (b, h) in sel
        ])
        in_maps.append({
            "xq_t": xq_t,
            "xk_t": xk_t,
            "cb": np.ascontiguousarray(cbs),
            "wt": wt,
        })
    return in_maps, pairs


def kernel(qs, ks, vs, qs_s, ks_s, fourier_W, qk_proj, a, _trace=False):
    from concourse.bass_utils import run_bass_kernel_spmd

    if "nc" not in _CACHE:
        _CACHE["nc"] = _build_kernel()
    nc = _CACHE["nc"]

    in_maps, pairs = _prep_inputs(
        np.asarray(qs), np.asarray(ks), np.asarray(vs), np.asarray(qs_s),
        np.asarray(ks_s), np.asarray(fourier_W), np.asarray(qk_proj),
        np.asarray(a))

    try:
        res = run_bass_kernel_spmd(
            nc, in_maps, core_ids=list(range(NCORES)), trace=_trace)
    except Exception:
        # the axon-tunneled devices occasionally throw a transient
        # NRT_EXEC_UNIT_UNRECOVERABLE; one retry has always recovered it
        res = run_bass_kernel_spmd(
            nc, in_maps, core_ids=list(range(NCORES)), trace=_trace)
    _CACHE["last_result"] = res

    full = np.empty((B, L, H, DV), np.float32)
    for core in range(NCORES):
        o = res.results[core]["out"]  # (PPC, 128, NCH, DV)
        for j, (b, h) in enumerate(pairs[PPC * core:PPC * (core + 1)]):
            full[b, :, h, :] = (
                o[j].transpose(1, 0, 2).reshape(L, DV).astype(np.float32))
    return full


# revision 24
# speedup vs baseline: 1.0043x; 1.0043x over previous
"""Distance-biased FAVOR+ fast attention on 8 Trainium2 NeuronCores.

Strategy: shard the 32 (batch, head) pairs across 8 cores (4 pairs/core).
Per pair, the device computes:
    qs'^T = exp(W~ @ Xq~^T)    (transposed layout, m on partitions)  [bf16]
    ks'   = exp(Xk~ @ W~^T)    (natural layout, L on partitions)     [bf16]
    buf1  = ks'^T @ c          (PSUM accumulation over 32 L-chunks)
    buf2  = qs' @ buf1         (natural layout), then num / max(den, EPS)
where Xq~/Xk~ are host-precomputed 73-dim augmented features
  [x*D^-0.25 (64), fourier dist features (8), h = 0.5*|x|^2 + ln(16)]
padded to 128 rows (full-partition DMAs run 5x faster than 73-row ones)
in fp16, and W~ = [qk_proj | -1] so exp(W~ X~^T) = exp(x W^T - h)/16 = phi(x).
The 1/sqrt(m) and -h terms ride inside the matmul; ACT exp needs no bias.
"""

import numpy as np
import ml_dtypes

B, L, H, D, DV = 4, 4096, 8, 64, 64
M = 256
DSH = 8             # per-head fourier feature dim (S // H)
KAUG = D + DSH + 1  # 73
NCORES = 8
PPC = (B * H) // NCORES  # pairs per core = 4
NCH = L // 128           # 32 chunks of 128 rows
EPS = 1e-6
LN16 = float(np.log(16.0))
TWO_PI = 2.0 * np.pi
NORM = float(D) ** -0.25

_CACHE = {}


def _build_kernel():
    import concourse.bass as bass
    import concourse.bacc as bacc
    import concourse.mybir as mybir
    import concourse.tile as tile

    f32 = mybir.dt.float32
    bf16 = mybir.dt.bfloat16
    fp16 = mybir.dt.float16
    Exp = mybir.ActivationFunctionType.Exp

    nc = bacc.Bacc("TRN2", debug=False, num_devices=NCORES)
    xq_t = nc.dram_tensor("xq_t", [PPC, 128, L], fp16, kind="ExternalInput")
    xk_t = nc.dram_tensor("xk_t", [PPC, 128, L], fp16, kind="ExternalInput")
    cb = nc.dram_tensor("cb", [PPC, 128, NCH, DV + 1], bf16, kind="ExternalInput")
    wt = nc.dram_tensor("wt", [128, M], fp16, kind="ExternalInput")
    out = nc.dram_tensor("out", [PPC, 128, NCH, DV], fp16, kind="ExternalOutput")

    with tile.TileContext(nc) as tc:
        with (
            tc.tile_pool(name="singles", bufs=1) as singles,
            tc.tile_pool(name="xin", bufs=3) as xin,
            tc.tile_pool(name="qp", bufs=2) as qp,
            tc.tile_pool(name="kpl", bufs=4) as kpl,
            tc.tile_pool(name="cpool", bufs=2) as cpool,
            tc.tile_pool(name="bsb", bufs=6) as bsb,
            tc.tile_pool(name="dvp", bufs=4) as dvp,
            tc.tile_pool(name="obp", bufs=3) as obp,
            tc.tile_pool(name="pslog", bufs=2, space="PSUM") as pslog,
            tc.tile_pool(name="psd", bufs=2, space="PSUM") as psd,
        ):
            wt_sb = singles.tile([128, M], fp16)
            nc.sync.dma_start(out=wt_sb, in_=wt[:, :])

            for i in range(PPC):
                xk = xin.tile([128, L], fp16, tag="xk")
                # bh0's first piece rides the empty HWDGE ring so the first
                # matmuls/exps are not stuck behind the SWDGE load burst
                keng = nc.sync if i == 0 else nc.gpsimd
                keng.dma_start(out=xk[:, :L // 4], in_=xk_t[i, :, :L // 4])
                c_sb = cpool.tile([128, NCH, DV + 1], bf16)
                nc.gpsimd.dma_start(out=c_sb, in_=cb[i])
                for piece in range(1, 4):
                    nc.gpsimd.dma_start(
                        out=xk[:, L // 4 * piece:L // 4 * (piece + 1)],
                        in_=xk_t[i, :, L // 4 * piece:L // 4 * (piece + 1)])
                xq = xin.tile([128, L], fp16, tag="xq")
                for piece in range(4):
                    nc.gpsimd.dma_start(
                        out=xq[:, L // 4 * piece:L // 4 * (piece + 1)],
                        in_=xq_t[i, :, L // 4 * piece:L // 4 * (piece + 1)])

                # ---- phase A-k + C: ks' natural; buf1 += ks'^T @ c per chunk
                b1ps = [
                    psd.tile([128, 7 * (DV + 1)], f32, tag="dout",
                             name=f"b1_{i}_{mi}")[:, :DV + 1]
                    for mi in range(2)
                ]
                nchunk = 0
                for g, gch in enumerate((2, 6, 6, 6, 6, 6)):
                    ps = pslog.tile([128, 1536], f32, tag="logits", name=f"psk_{i}_{g}")
                    for j in range(gch):
                        n = nchunk + j
                        nc.tensor.matmul(
                            ps[:, 256 * j:256 * (j + 1)],
                            lhsT=xk[:, 128 * n:128 * (n + 1)],
                            rhs=wt_sb,
                            start=True, stop=True,
                        )
                    kp = kpl.tile([128, 1536], bf16, tag="kp", name=f"kp_{i}_{g}")
                    nc.scalar.activation(
                        out=kp[:, :256 * gch], in_=ps[:, :256 * gch], func=Exp,
                        bias=0.0, scale=1.0)
                    for j in range(gch):
                        n = nchunk + j
                        for mi in range(2):
                            nc.tensor.matmul(
                                b1ps[mi],
                                lhsT=kp[:, 256 * j + 128 * mi:256 * j + 128 * (mi + 1)],
                                rhs=c_sb[:, n, :],
                                start=(n == 0), stop=(n == NCH - 1),
                            )
                    nchunk += gch
                buf1 = []
                for mi in range(2):
                    bsl = bsb.tile([128, DV + 1], bf16, tag=f"b1sb_{mi}",
                                   name=f"b1sb_{i}_{mi}")
                    nc.vector.tensor_copy(out=bsl, in_=b1ps[mi])
                    buf1.append(bsl)

                # ---- phase A-q: qs'^T = exp(W~ Xq~^T); g outer, mi inner so
                # phase D's chunk groups unblock as early as possible
                qpT = [
                    qp.tile([128, L], bf16, tag=f"qpT{mi}", name=f"qpT{mi}_{i}")
                    for mi in range(2)
                ]
                col = 0
                for g, w in enumerate((1536, 1536, 1024)):
                    for mi in range(2):
                        ps = pslog.tile([128, 1536], f32, tag="logits",
                                        name=f"psq_{i}_{mi}_{g}")
                        for n2 in range(w // 512):
                            nc.tensor.matmul(
                                ps[:, 512 * n2:512 * (n2 + 1)],
                                lhsT=wt_sb[:, 128 * mi:128 * (mi + 1)],
                                rhs=xq[:, col + 512 * n2:col + 512 * (n2 + 1)],
                                start=True, stop=True,
                            )
                        nc.scalar.activation(
                            out=qpT[mi][:, col:col + w], in_=ps[:, :w],
                            func=Exp, bias=0.0, scale=1.0,
                        )
                    col += w

                # ---- phase D: buf2 = qs' @ buf1 per L-chunk; divide; store
                ob = obp.tile([128, NCH * DV], fp16, tag="ob", name=f"ob_{i}")
                n0 = 0
                for gi, cnt in enumerate((7, 7, 7, 7, 4)):
                    dps = psd.tile([128, 7 * (DV + 1)], f32, tag="dout",
                                   name=f"dps_{i}_{gi}")
                    dps3 = dps.rearrange("p (s e) -> p s e", e=DV + 1)
                    for s in range(cnt):
                        n = n0 + s
                        for mi in range(2):
                            nc.tensor.matmul(
                                dps3[:, s, :],
                                lhsT=qpT[mi][:, 128 * n:128 * (n + 1)],
                                rhs=buf1[mi],
                                start=(mi == 0), stop=(mi == 1),
                            )
                    den = dvp.tile([128, 7], f32, tag="den", name=f"den_{i}_{gi}")
                    nc.vector.tensor_scalar_max(
                        out=den[:, :cnt], in0=dps3[:, :cnt, DV], scalar1=EPS)
                    nc.vector.reciprocal(out=den[:, :cnt], in_=den[:, :cnt])
                    den_sl = den[:, :cnt]
                    den_bc = bass.AP(
                        tensor=den_sl.tensor, offset=den_sl.offset,
                        ap=[den_sl.ap[0], den_sl.ap[1], [0, DV]])
                    nc.vector.tensor_tensor(
                        out=ob[:, DV * n0:DV * (n0 + cnt)].rearrange(
                            "p (s e) -> p s e", e=DV),
                        in0=dps3[:, :cnt, 0:DV],
                        in1=den_bc,
                        op=mybir.AluOpType.mult,
                    )
                    oeng = nc.sync if i == PPC - 1 else nc.gpsimd
                    ob3 = ob.rearrange("p (n e) -> p n e", e=DV)
                    oeng.dma_start(out=out[i, :, n0:n0 + cnt, :],
                                   in_=ob3[:, n0:n0 + cnt, :])
                    n0 += cnt
    nc.compile()
    return nc


def _prep_inputs(qs, ks, vs, qs_s, ks_s, fourier_W, qk_proj, a):
    """Host-side: fourier features, augmentation, transposes, per-core split."""
    pq = TWO_PI * (qs_s @ fourier_W)       # (B, L, 32)
    pk = TWO_PI * (ks_s @ fourier_W)
    embq = np.concatenate([np.sin(pq), np.cos(pq)], axis=-1).astype(np.float32)
    embk = np.concatenate([np.sin(pk), np.cos(pk)], axis=-1).astype(np.float32)
    qs_sp = (a * embq.reshape(B, L, H, DSH)).astype(np.float32)  # (B,L,H,8)
    ks_sp = embk.reshape(B, L, H, DSH)

    xq = np.concatenate([qs * NORM, qs_sp], axis=-1)  # (B,L,H,72)
    xk = np.concatenate([ks * NORM, ks_sp], axis=-1)
    hq = 0.5 * np.sum(np.square(xq), axis=-1, keepdims=True) + LN16
    hk = 0.5 * np.sum(np.square(xk), axis=-1, keepdims=True) + LN16
    xq = np.concatenate([xq, hq], axis=-1)  # (B,L,H,73)
    xk = np.concatenate([xk, hk], axis=-1)

    c = np.concatenate([vs, np.ones((B, L, H, 1), vs.dtype)], axis=-1)
    c = c.astype(ml_dtypes.bfloat16)        # (B,L,H,65)

    wt_f = np.concatenate(
        [qk_proj, -np.ones((M, 1), np.float32)], axis=1).T  # (73, 256)
    wt = np.zeros((128, M), np.float16)
    wt[:KAUG] = wt_f

    pairs = [(b, h) for b in range(B) for h in range(H)]
    in_maps = []
    for core in range(NCORES):
        sel = pairs[PPC * core:PPC * (core + 1)]
        xq_t = np.zeros((PPC, 128, L), np.float16)
        xk_t = np.zeros((PPC, 128, L), np.float16)
        for j, (b, h) in enumerate(sel):
            xq_t[j, :KAUG] = xq[b, :, h, :].T
            xk_t[j, :KAUG] = xk[b, :, h, :].T
        cbs = np.stack([
            c[b, :, h, :].reshape(NCH, 128, DV + 1).transpose(1, 0, 2)
            for (b, h) in sel
        ])
        in_maps.append({
            "xq_t": xq_t,
            "xk_t": xk_t,
            "cb": np.ascontiguousarray(cbs),
            "wt": wt,
        })
    return in_maps, pairs


def kernel(qs, ks, vs, qs_s, ks_s, fourier_W, qk_proj, a, _trace=False):
    from concourse.bass_utils import run_bass_kernel_spmd

    if "nc" not in _CACHE:
        _CACHE["nc"] = _build_kernel()
    nc = _CACHE["nc"]

    in_maps, pairs = _prep_inputs(
        np.asarray(qs), np.asarray(ks), np.asarray(vs), np.asarray(qs_s),
        np.asarray(ks_s), np.asarray(fourier_W), np.asarray(qk_proj),
        np.asarray(a))

    try:
        res = run_bass_kernel_spmd(
            nc, in_maps, core_ids=list(range(NCORES)), trace=_trace)
    except Exception:
        # the axon-tunneled devices occasionally throw a transient
        # NRT_EXEC_UNIT_UNRECOVERABLE; one retry has always recovered it
        res = run_bass_kernel_spmd(
            nc, in_maps, core_ids=list(range(NCORES)), trace=_trace)
    _CACHE["last_result"] = res

    full = np.empty((B, L, H, DV), np.float32)
    for core in range(NCORES):
        o = res.results[core]["out"]  # (PPC, 128, NCH, DV)
        for j, (b, h) in enumerate(pairs[PPC * core:PPC * (core + 1)]):
            full[b, :, h, :] = (
                o[j].transpose(1, 0, 2).reshape(L, DV).astype(np.float32))
    return full
